# revision 31
# baseline (speedup 1.0000x reference)
"""Trainium2 Bass kernel for nn_CGCoupler (segment_reduce).

The CG tables decompose into 147 block-ops out[bo] += c * x1[b1] * x2[b2]
over 64-aligned blocks = (l, m) spherical-harmonic slots (block = l^2+l+m),
metadata=[64,64,64,64]. The coefficients obey the mirror symmetry
c(b1,b2,bo) = +-c(b2,b1,bo), so mirrored op pairs collapse into single
terms over symmetrized products S = g12 + g21^T / A = g12 - g21^T:
147 ops -> 78 terms, max 6 per output segment.

Per core (512 rows = 4 row-groups of 128 partitions), bf16 compute:
 1. inputs pre-converted to bf16 on the host (halves input DMA bytes,
    no on-chip casts)
 2. products+sym into a 73-entry term space (few large broadcast DVE ops)
 3. expand+scale: gather terms -> j-major segment slots (6/segment) fused
    with the cg multiply, as arithmetic-progression runs over the term
    space (plus one strided instruction for the whole S0 family)
 4. segment reduce: j-major layout makes each tree level (6->3->2->1) a
    single contiguous bf16 add; the final add outputs fp32 directly
 5. DMA out (no separate output cast)
"""
import numpy as np

N_CORES = 8
ROWS_PER_CORE = 512
D = 1024
PAD = 6
NSEG = 16
SLOTS = NSEG * PAD      # 96
NTERM = 73              # term-space entries

# diagonal ops (a, bo) with b1 == b2 == a
DIAG = [(0, 0), (1, 0), (1, 6), (1, 8), (2, 0), (2, 6), (3, 0), (3, 6), (3, 8)]
# mirrored op pairs (a, b, bo, sigma), canonical a < b:
# c(a,b,bo) = sigma * c(b,a,bo); term = c(a,b,bo) * (g[a,b] + sigma*g[b,a])
SYM = [
    (0, 1, 1, 1), (0, 2, 2, 1), (0, 3, 3, 1), (0, 4, 4, 1), (0, 5, 5, 1), (0, 6, 6, 1),
    (0, 7, 7, 1), (0, 8, 8, 1), (0, 9, 9, 1), (0, 10, 10, 1), (0, 11, 11, 1), (0, 12, 12, 1),
    (0, 13, 13, 1), (0, 14, 14, 1), (0, 15, 15, 1), (1, 2, 3, -1), (1, 2, 5, 1), (1, 3, 2, -1),
    (1, 3, 4, 1), (1, 4, 3, 1), (1, 4, 5, -1), (1, 4, 13, 1), (1, 4, 15, 1), (1, 5, 2, 1),
    (1, 5, 4, -1), (1, 5, 12, 1), (1, 5, 14, 1), (1, 6, 1, 1), (1, 6, 7, -1), (1, 6, 11, 1),
    (1, 7, 6, -1), (1, 7, 8, -1), (1, 7, 10, 1), (1, 8, 1, 1), (1, 8, 7, -1), (1, 8, 9, 1),
    (1, 8, 11, 1), (2, 3, 1, -1), (2, 3, 7, 1), (2, 4, 8, -1), (2, 4, 10, 1), (2, 5, 1, 1),
    (2, 5, 7, -1), (2, 5, 11, 1), (2, 6, 2, 1), (2, 6, 12, 1), (2, 7, 3, 1), (2, 7, 5, -1),
    (2, 7, 13, 1), (2, 8, 4, -1), (2, 8, 14, 1), (3, 4, 1, 1), (3, 4, 7, -1), (3, 4, 9, 1),
    (3, 4, 11, 1), (3, 5, 6, -1), (3, 5, 8, -1), (3, 5, 10, 1), (3, 6, 3, 1), (3, 6, 5, -1),
    (3, 6, 13, 1), (3, 7, 2, 1), (3, 7, 4, -1), (3, 7, 12, 1), (3, 7, 14, 1), (3, 8, 3, 1),
    (3, 8, 5, -1), (3, 8, 13, 1), (3, 8, 15, 1),
]

# term-space layout (entry offsets)
E_G00 = 0      # 1: x1[0]*x2[0]
E_G11 = 1      # 9: l1=l2=1 raster (m1*3+m2); diag entries used directly
E_S01 = 10     # 3
E_S02 = 13     # 5
E_S03 = 18     # 7
E_S11 = 25     # 9 raster (entries with m1<m2 used)
E_A11 = 34     # 9
E_S12 = 43     # 15 raster (m1*5+m2) for (l=1, l=2) canonical pairs
E_A12 = 58     # 15


def _lm(b):
    l = int(np.sqrt(b))
    return l, b - l * l - l


def _term_entry(a, b, sigma):
    """Term-space entry for canonical pair (a<b) with sign sigma."""
    la, ma = _lm(a)
    lb, mb = _lm(b)
    if la == 0:
        assert sigma == 1
        return {1: E_S01, 2: E_S02, 3: E_S03}[lb] + (mb + lb)
    if la == 1 and lb == 1:
        base = E_S11 if sigma == 1 else E_A11
        return base + (ma + 1) * 3 + (mb + 1)
    assert la == 1 and lb == 2
    base = E_S12 if sigma == 1 else E_A12
    return base + (ma + 1) * 5 + (mb + 2)


def _diag_entry(a):
    if a == 0:
        return E_G00
    return E_G11 + (a - 1) * 4   # raster diag of g11


def _ap_single(es, L):
    """(e0, d) for one AP of length L whose value set contains all of the
    sorted entries `es`, values within [0, NTERM); None if impossible.
    Unused positions become wildcard pads (crep = 0)."""
    import math
    if not es:
        return (0, 0)
    if len(es) > L:
        return None
    if len(es) == 1:
        return (es[0], 0)
    g = 0
    for a, b in zip(es, es[1:]):
        g = math.gcd(g, b - a)
    span = es[-1] - es[0]
    for d in range(1, NTERM):
        if g % d or span // d > L - 1:
            continue
        for k in range(L - span // d):
            e0 = es[0] - k * d
            if e0 >= 0 and e0 + (L - 1) * d < NTERM:
                return (e0, d)
    return None


def _ap_cover(entries, nslots):
    """Partition nslots consecutive slots into the fewest APs covering all
    entries (wildcard pads allowed inside runs). Returns [(L, e0, d)]."""
    E = sorted(entries)
    n = len(E)
    r = _ap_single(E, nslots)
    if r is not None:
        return [(nslots, r[0], r[1])]
    for k in range(1, nslots):
        for mask in range(1 << n):
            A = [E[i] for i in range(n) if mask >> i & 1]
            B = [E[i] for i in range(n) if not (mask >> i & 1)]
            ra = _ap_single(A, k)
            rb = _ap_single(B, nslots - k)
            if ra and rb:
                return [(k, ra[0], ra[1]), (nslots - k, rb[0], rb[1])]
    for k1 in range(1, nslots - 1):
        for k2 in range(k1 + 1, nslots):
            for mask1 in range(1 << n):
                A = [E[i] for i in range(n) if mask1 >> i & 1]
                rem = [i for i in range(n) if not (mask1 >> i & 1)]
                ra = _ap_single(A, k1)
                if not ra:
                    continue
                for mask2 in range(1 << len(rem)):
                    B = [E[rem[i]] for i in range(len(rem)) if mask2 >> i & 1]
                    C = [E[rem[i]] for i in range(len(rem))
                         if not (mask2 >> i & 1)]
                    rb = _ap_single(B, k2 - k1)
                    rc = _ap_single(C, nslots - k2)
                    if rb and rc:
                        return [(k1, ra[0], ra[1]), (k2 - k1, rb[0], rb[1]),
                                (nslots - k2, rc[0], rc[1])]
    raise AssertionError("no 3-run cover")


def _plan():
    """runs = [(slot0, len, entry0, dentry)] for per-segment AP runs; the
    S0-family (one term per segment 1..15, entries E_S01+bo-1, placed at
    slot 0 of each segment) is emitted separately as strided-slot
    instructions. Pads are appended to runs when the extended entry stays
    in [0, NTERM), else get zero-coeff dg=0 runs.
    slot_key[slot] = (a, b, bo) runtime-table key."""
    segterms = {bo: [] for bo in range(NSEG)}
    for a, bo in DIAG:
        segterms[bo].append((_diag_entry(a), (a, a, bo)))
    for a, b, bo, s in SYM:
        if a == 0:
            continue   # S0-family handled as a strided-slot instruction
        segterms[bo].append((_term_entry(a, b, s), (a, b, bo)))
    # j-major slot ids: slot(bo, j) = grp*48 + j*8 + (bo - grp*8).
    # Tree levels then read/write fully contiguous 24/8-slot blocks, and
    # expand runs write slot strides of 8.
    def sid(bo, j):
        grp = bo // 8
        return grp * 48 + j * 8 + (bo - grp * 8)

    runs = []   # (bo, j0, len, entry0, dentry)
    slot_key = {}
    for bo in range(1, NSEG):
        slot_key[sid(bo, 0)] = (0, bo, bo)
    for bo in range(NSEG):
        terms = segterms[bo]
        j = 0 if bo == 0 else 1   # j=0 taken by the S0 term
        assert len(terms) <= PAD - j
        bypos = dict(terms)
        for (L, e0, de) in _ap_cover(list(bypos), PAD - j):
            runs.append((bo, j, L, e0, de))
            for t in range(L):
                e = e0 + t * de
                if e in bypos:
                    slot_key[sid(bo, j + t)] = bypos.pop(e)
            j += L
        assert not bypos and j == PAD
    return runs, slot_key


RUNS, SLOT_KEY = _plan()

_CACHE = {}


def _build():
    from concourse import bacc, mybir
    import concourse.tile as tile

    f32 = mybir.dt.float32
    bf16 = mybir.dt.bfloat16
    ALU = mybir.AluOpType
    G = ROWS_PER_CORE // 128

    nc = bacc.Bacc("TRN2", target_bir_lowering=False)
    x1_d = nc.dram_tensor("x1", [ROWS_PER_CORE, D], bf16, kind="ExternalInput")
    x2_d = nc.dram_tensor("x2", [ROWS_PER_CORE, D], bf16, kind="ExternalInput")
    cg_d = nc.dram_tensor("cgslot", [128, SLOTS * 64], bf16, kind="ExternalInput")
    out_d = nc.dram_tensor("out", [ROWS_PER_CORE, D], f32, kind="ExternalOutput")

    with tile.TileContext(nc) as tc:
        with (
            tc.tile_pool(name="const", bufs=1) as constp,
            tc.tile_pool(name="scratch", bufs=1) as scrp,
            tc.tile_pool(name="big", bufs=1) as bigp,
            tc.tile_pool(name="tree", bufs=1) as treep,
        ):
            crep = constp.tile([128, SLOTS * 64], bf16)
            crep3 = crep[:].rearrange("p (s n) -> p s n", s=SLOTS)

            x1b = constp.tile([128, G, D], bf16)
            x2b = constp.tile([128, G, D], bf16)
            term = bigp.tile([128, G, NTERM, 64], bf16)
            g12 = scrp.tile([128, G, 15, 64], bf16)
            g21 = scrp.tile([128, G, 15, 64], bf16)
            g0x = scrp.tile([128, G, 15, 64], bf16)
            gx0 = scrp.tile([128, G, 15, 64], bf16)

            # inputs arrive pre-converted to bf16 (host does the cast), so
            # DMAs land directly in x1b/x2b — no staging, no Act casts.
            # Tile 0 cols 64:576 (blocks 1..8, the only per-tile product
            # inputs) ship first so DVE starts early; blocks 0 and 9..15
            # (used only by the late g-merged instructions) ship after the
            # other tiles. crep (first needed at expand) goes last.
            nc.sync.dma_start(x1b[:, 0, 64:576], x1_d[0:128, 64:576])
            nc.gpsimd.dma_start(x2b[:, 0, 64:576], x2_d[0:128, 64:576])
            for t in range(1, G):
                nc.sync.dma_start(x1b[:, t], x1_d[t*128:(t+1)*128])
                nc.gpsimd.dma_start(x2b[:, t], x2_d[t*128:(t+1)*128])
            nc.sync.dma_start(x1b[:, 0, 0:64], x1_d[0:128, 0:64])
            nc.sync.dma_start(x1b[:, 0, 576:1024], x1_d[0:128, 576:1024])
            nc.gpsimd.dma_start(x2b[:, 0, 0:64], x2_d[0:128, 0:64])
            nc.gpsimd.dma_start(x2b[:, 0, 576:1024], x2_d[0:128, 576:1024])
            nc.gpsimd.dma_start(crep[:], cg_d[:])

            for t in range(G):
                a1 = x1b[:, t].rearrange("p (b n) -> p b n", b=16)
                a2 = x2b[:, t].rearrange("p (b n) -> p b n", b=16)
                # g11 raster [3,3]
                t11 = term[:, t, E_G11:E_G11 + 9, :].rearrange(
                    "p (a b) n -> p a b n", a=3)
                nc.vector.tensor_mul(
                    t11,
                    a1[:, 1:4, :].unsqueeze(2).to_broadcast([128, 3, 3, 64]),
                    a2[:, 1:4, :].unsqueeze(1).to_broadcast([128, 3, 3, 64]))
                # g12 / g21
                v12 = g12[:, t].rearrange("p (a b) n -> p a b n", a=3)
                v21 = g21[:, t].rearrange("p (a b) n -> p a b n", a=5)
                nc.vector.tensor_mul(
                    v12,
                    a1[:, 1:4, :].unsqueeze(2).to_broadcast([128, 3, 5, 64]),
                    a2[:, 4:9, :].unsqueeze(1).to_broadcast([128, 3, 5, 64]))
                nc.vector.tensor_mul(
                    v21,
                    a1[:, 4:9, :].unsqueeze(2).to_broadcast([128, 5, 3, 64]),
                    a2[:, 1:4, :].unsqueeze(1).to_broadcast([128, 5, 3, 64]))
                # S11/A11 = g11 +- g11^T
                t11v = term[:, t, E_G11:E_G11 + 9, :].rearrange(
                    "p (a b) n -> p a b n", a=3)
                s11 = term[:, t, E_S11:E_S11 + 9, :].rearrange(
                    "p (a b) n -> p a b n", a=3)
                a11 = term[:, t, E_A11:E_A11 + 9, :].rearrange(
                    "p (a b) n -> p a b n", a=3)
                nc.vector.tensor_tensor(s11, t11v, t11v.transpose([0, 2, 1, 3]),
                                        op=ALU.add)
                nc.vector.tensor_tensor(a11, t11v, t11v.transpose([0, 2, 1, 3]),
                                        op=ALU.subtract)
                s12 = term[:, t, E_S12:E_S12 + 15, :].rearrange(
                    "p (a b) n -> p a b n", a=3)
                a12 = term[:, t, E_A12:E_A12 + 15, :].rearrange(
                    "p (a b) n -> p a b n", a=3)
                nc.vector.tensor_tensor(s12, v12, v21.transpose([0, 2, 1, 3]),
                                        op=ALU.add)
                nc.vector.tensor_tensor(a12, v12, v21.transpose([0, 2, 1, 3]),
                                        op=ALU.subtract)

            # g-merged: g00 and S0k = x1[0]*x2[blk] + x1[blk]*x2[0]
            b1v = x1b[:].rearrange("p g (b n) -> p g b n", b=16)
            b2v = x2b[:].rearrange("p g (b n) -> p g b n", b=16)
            nc.vector.tensor_mul(term[:, :, E_G00:E_G00 + 1, :],
                                 b1v[:, :, 0:1, :], b2v[:, :, 0:1, :])
            nc.vector.tensor_mul(
                g0x[:], b1v[:, :, 0:1, :].to_broadcast([128, G, 15, 64]),
                b2v[:, :, 1:16, :])
            nc.vector.tensor_mul(
                gx0[:], b1v[:, :, 1:16, :],
                b2v[:, :, 0:1, :].to_broadcast([128, G, 15, 64]))
            nc.vector.tensor_tensor(term[:, :, E_S01:E_S01 + 15, :],
                                    g0x[:], gx0[:], op=ALU.add)

            # expand+scale then tree, one 8-segment group at a time.
            # j-major slots: in-group slot = j*8 + seg_off, so tree levels
            # read fully contiguous 24/8-slot blocks.
            for grp in range(2):
                seg0 = grp * 8
                lo = grp * 48
                sp = bigp.tile([128, G, 8 * PAD, 64], bf16, tag="sp")
                t1 = treep.tile([128, G, 3 * 8, 64], bf16, tag="t1")
                u = treep.tile([128, G, 8, 64], bf16, tag="u")
                res = treep.tile([128, G, 8, 64], f32, tag="res")

                # S0-family: j=0 slots, entries contiguous (E_S01 + bo - 1)
                if grp == 0:
                    nseg0, efam, sfam = 7, E_S01, 1        # segments 1..7
                else:
                    nseg0, efam, sfam = 8, E_S01 + 7, 0    # segments 8..15
                cfam = crep3[:, lo + sfam:lo + sfam + nseg0, :].unsqueeze(
                    1).to_broadcast([128, G, nseg0, 64])
                nc.vector.tensor_mul(
                    sp[:, :, sfam:sfam + nseg0, :],
                    term[:, :, efam:efam + nseg0, :], cfam)

                for (bo, j0, ln, e0, de) in RUNS:
                    if not (seg0 <= bo < seg0 + 8):
                        continue
                    if ln == 1 or de == 0:
                        gsl = term[:, :, e0:e0 + 1, :]
                        if ln > 1:
                            gsl = gsl.to_broadcast([128, G, ln, 64])
                    elif de > 0:
                        gsl = term[:, :, e0:e0 + (ln - 1) * de + 1:de, :]
                    else:
                        stop = e0 + (ln - 1) * de - 1
                        gsl = term[:, :, e0:(stop if stop >= 0 else None):de, :]
                    s0 = j0 * 8 + (bo - seg0)   # in-group slot, stride 8
                    send = s0 + (ln - 1) * 8 + 1
                    csl = crep3[:, lo + s0:lo + send:8, :].unsqueeze(
                        1).to_broadcast([128, G, ln, 64])
                    nc.vector.tensor_mul(sp[:, :, s0:send:8, :], gsl, csl)

                # tree 6->3->2->1 over j-major blocks: 3 instructions
                nc.vector.tensor_tensor(
                    t1[:], sp[:, :, 0:24, :], sp[:, :, 24:48, :], op=ALU.add)
                nc.vector.tensor_tensor(
                    u[:], t1[:, :, 0:8, :], t1[:, :, 8:16, :], op=ALU.add)
                # final add in fp32: output-ready, no separate cast
                nc.vector.tensor_tensor(
                    res[:], u[:], t1[:, :, 16:24, :], op=ALU.add)

                # alternate DMA issue queues to halve tail serialization
                for g in range(G):
                    eng = nc.sync if g % 2 == 0 else nc.scalar
                    eng.dma_start(
                        out_d[g*128:(g+1)*128, seg0*64:(seg0 + 8)*64],
                        res[:, g])

    nc.compile()
    return nc


def _get_nc():
    if "nc" not in _CACHE:
        _CACHE["nc"] = _build()
    return _CACHE["nc"]


def _in_maps(np_inputs):
    import ml_dtypes
    # host-side fp32 -> bf16 conversion: halves input DMA bytes and removes
    # all on-chip casts (same rounding the Activation engine would apply)
    x1 = np.asarray(np_inputs["x1"], dtype=np.float32).astype(ml_dtypes.bfloat16)
    x2 = np.asarray(np_inputs["x2"], dtype=np.float32).astype(ml_dtypes.bfloat16)
    cg = np.asarray(np_inputs["cg_tilde"], dtype=np.float32).reshape(-1, 64)
    rid1 = np.asarray(np_inputs["repids_in1"]).reshape(-1, 64)[:, 0] // 64
    rid2 = np.asarray(np_inputs["repids_in2"]).reshape(-1, 64)[:, 0] // 64
    rido = np.asarray(np_inputs["repids_out"]).reshape(-1, 64)[:, 0] // 64

    table = {}
    for k in range(cg.shape[0]):
        table[(int(rid1[k]), int(rid2[k]), int(rido[k]))] = cg[k, 0]
    cg_slot = np.zeros(SLOTS, dtype=np.float32)
    for slot, key in SLOT_KEY.items():
        cg_slot[slot] = table[key]
    cg_full = np.broadcast_to(cg_slot[:, None], (SLOTS, 64)).reshape(1, -1)
    cg_full = np.ascontiguousarray(
        np.broadcast_to(cg_full, (128, SLOTS * 64))).astype(ml_dtypes.bfloat16)

    n = x1.shape[0]
    rows = n // N_CORES
    in_maps = []
    for k in range(N_CORES):
        sl = slice(k * rows, (k + 1) * rows)
        in_maps.append({
            "x1": np.ascontiguousarray(x1[sl]),
            "x2": np.ascontiguousarray(x2[sl]),
            "cgslot": cg_full,
        })
    return in_maps


def kernel(x1, x2, cg_tilde, repids_in1, repids_in2, repids_out, out_dim):
    from concourse.bass_utils import run_bass_kernel_spmd

    nc = _get_nc()
    in_maps = _in_maps({
        "x1": x1, "x2": x2, "cg_tilde": cg_tilde, "repids_in1": repids_in1,
        "repids_in2": repids_in2, "repids_out": repids_out,
    })
    res = run_bass_kernel_spmd(nc, in_maps, core_ids=list(range(N_CORES)))
    out = np.concatenate([res.results[k]["out"] for k in range(N_CORES)], axis=0)
    return out


# revision 32
# speedup vs baseline: 1.1934x; 1.1934x over previous
"""Trainium2 Bass kernel for nn_CGCoupler (segment_reduce).

The CG tables decompose into 147 block-ops out[bo] += c * x1[b1] * x2[b2]
over 64-aligned blocks = (l, m) spherical-harmonic slots (block = l^2+l+m),
metadata=[64,64,64,64]. The coefficients obey the mirror symmetry
c(b1,b2,bo) = +-c(b2,b1,bo), so mirrored op pairs collapse into single
terms over symmetrized products S = g12 + g21^T / A = g12 - g21^T:
147 ops -> 78 terms, max 6 per output segment.

Per core (512 rows = 4 row-groups of 128 partitions), bf16 compute:
 1. inputs pre-converted to bf16 on the host (halves input DMA bytes,
    no on-chip casts)
 2. products+sym into a 73-entry term space (few large broadcast DVE ops)
 3. expand+scale: gather terms -> j-major segment slots (6/segment) fused
    with the cg multiply, as arithmetic-progression runs over the term
    space (plus one strided instruction for the whole S0 family)
 4. segment reduce: j-major layout makes each tree level (6->3->2->1) a
    single contiguous bf16 add; the final add outputs fp32 directly
 5. DMA out (no separate output cast)
"""
import numpy as np

N_CORES = 8
ROWS_PER_CORE = 512
D = 1024
PAD = 6
NSEG = 16
SLOTS = NSEG * PAD      # 96
NTERM = 73              # term-space entries

# diagonal ops (a, bo) with b1 == b2 == a
DIAG = [(0, 0), (1, 0), (1, 6), (1, 8), (2, 0), (2, 6), (3, 0), (3, 6), (3, 8)]
# mirrored op pairs (a, b, bo, sigma), canonical a < b:
# c(a,b,bo) = sigma * c(b,a,bo); term = c(a,b,bo) * (g[a,b] + sigma*g[b,a])
SYM = [
    (0, 1, 1, 1), (0, 2, 2, 1), (0, 3, 3, 1), (0, 4, 4, 1), (0, 5, 5, 1), (0, 6, 6, 1),
    (0, 7, 7, 1), (0, 8, 8, 1), (0, 9, 9, 1), (0, 10, 10, 1), (0, 11, 11, 1), (0, 12, 12, 1),
    (0, 13, 13, 1), (0, 14, 14, 1), (0, 15, 15, 1), (1, 2, 3, -1), (1, 2, 5, 1), (1, 3, 2, -1),
    (1, 3, 4, 1), (1, 4, 3, 1), (1, 4, 5, -1), (1, 4, 13, 1), (1, 4, 15, 1), (1, 5, 2, 1),
    (1, 5, 4, -1), (1, 5, 12, 1), (1, 5, 14, 1), (1, 6, 1, 1), (1, 6, 7, -1), (1, 6, 11, 1),
    (1, 7, 6, -1), (1, 7, 8, -1), (1, 7, 10, 1), (1, 8, 1, 1), (1, 8, 7, -1), (1, 8, 9, 1),
    (1, 8, 11, 1), (2, 3, 1, -1), (2, 3, 7, 1), (2, 4, 8, -1), (2, 4, 10, 1), (2, 5, 1, 1),
    (2, 5, 7, -1), (2, 5, 11, 1), (2, 6, 2, 1), (2, 6, 12, 1), (2, 7, 3, 1), (2, 7, 5, -1),
    (2, 7, 13, 1), (2, 8, 4, -1), (2, 8, 14, 1), (3, 4, 1, 1), (3, 4, 7, -1), (3, 4, 9, 1),
    (3, 4, 11, 1), (3, 5, 6, -1), (3, 5, 8, -1), (3, 5, 10, 1), (3, 6, 3, 1), (3, 6, 5, -1),
    (3, 6, 13, 1), (3, 7, 2, 1), (3, 7, 4, -1), (3, 7, 12, 1), (3, 7, 14, 1), (3, 8, 3, 1),
    (3, 8, 5, -1), (3, 8, 13, 1), (3, 8, 15, 1),
]

# term-space layout (entry offsets)
E_G00 = 0      # 1: x1[0]*x2[0]
E_G11 = 1      # 9: l1=l2=1 raster (m1*3+m2); diag entries used directly
E_S01 = 10     # 3
E_S02 = 13     # 5
E_S03 = 18     # 7
E_S11 = 25     # 9 raster (entries with m1<m2 used)
E_A11 = 34     # 9
E_S12 = 43     # 15 raster (m1*5+m2) for (l=1, l=2) canonical pairs
E_A12 = 58     # 15


def _lm(b):
    l = int(np.sqrt(b))
    return l, b - l * l - l


def _term_entry(a, b, sigma):
    """Term-space entry for canonical pair (a<b) with sign sigma."""
    la, ma = _lm(a)
    lb, mb = _lm(b)
    if la == 0:
        assert sigma == 1
        return {1: E_S01, 2: E_S02, 3: E_S03}[lb] + (mb + lb)
    if la == 1 and lb == 1:
        base = E_S11 if sigma == 1 else E_A11
        return base + (ma + 1) * 3 + (mb + 1)
    assert la == 1 and lb == 2
    base = E_S12 if sigma == 1 else E_A12
    return base + (ma + 1) * 5 + (mb + 2)


def _diag_entry(a):
    if a == 0:
        return E_G00
    return E_G11 + (a - 1) * 4   # raster diag of g11


def _ap_single(es, L):
    """(e0, d) for one AP of length L whose value set contains all of the
    sorted entries `es`, values within [0, NTERM); None if impossible.
    Unused positions become wildcard pads (crep = 0)."""
    import math
    if not es:
        return (0, 0)
    if len(es) > L:
        return None
    if len(es) == 1:
        return (es[0], 0)
    g = 0
    for a, b in zip(es, es[1:]):
        g = math.gcd(g, b - a)
    span = es[-1] - es[0]
    for d in range(1, NTERM):
        if g % d or span // d > L - 1:
            continue
        for k in range(L - span // d):
            e0 = es[0] - k * d
            if e0 >= 0 and e0 + (L - 1) * d < NTERM:
                return (e0, d)
    return None


def _ap_cover(entries, nslots):
    """Partition nslots consecutive slots into the fewest APs covering all
    entries (wildcard pads allowed inside runs). Returns [(L, e0, d)]."""
    E = sorted(entries)
    n = len(E)
    r = _ap_single(E, nslots)
    if r is not None:
        return [(nslots, r[0], r[1])]
    for k in range(1, nslots):
        for mask in range(1 << n):
            A = [E[i] for i in range(n) if mask >> i & 1]
            B = [E[i] for i in range(n) if not (mask >> i & 1)]
            ra = _ap_single(A, k)
            rb = _ap_single(B, nslots - k)
            if ra and rb:
                return [(k, ra[0], ra[1]), (nslots - k, rb[0], rb[1])]
    for k1 in range(1, nslots - 1):
        for k2 in range(k1 + 1, nslots):
            for mask1 in range(1 << n):
                A = [E[i] for i in range(n) if mask1 >> i & 1]
                rem = [i for i in range(n) if not (mask1 >> i & 1)]
                ra = _ap_single(A, k1)
                if not ra:
                    continue
                for mask2 in range(1 << len(rem)):
                    B = [E[rem[i]] for i in range(len(rem)) if mask2 >> i & 1]
                    C = [E[rem[i]] for i in range(len(rem))
                         if not (mask2 >> i & 1)]
                    rb = _ap_single(B, k2 - k1)
                    rc = _ap_single(C, nslots - k2)
                    if rb and rc:
                        return [(k1, ra[0], ra[1]), (k2 - k1, rb[0], rb[1]),
                                (nslots - k2, rc[0], rc[1])]
    raise AssertionError("no 3-run cover")


def _plan():
    """runs = [(slot0, len, entry0, dentry)] for per-segment AP runs; the
    S0-family (one term per segment 1..15, entries E_S01+bo-1, placed at
    slot 0 of each segment) is emitted separately as strided-slot
    instructions. Pads are appended to runs when the extended entry stays
    in [0, NTERM), else get zero-coeff dg=0 runs.
    slot_key[slot] = (a, b, bo) runtime-table key."""
    segterms = {bo: [] for bo in range(NSEG)}
    for a, bo in DIAG:
        segterms[bo].append((_diag_entry(a), (a, a, bo)))
    for a, b, bo, s in SYM:
        if a == 0:
            continue   # S0-family handled as a strided-slot instruction
        segterms[bo].append((_term_entry(a, b, s), (a, b, bo)))
    # j-major slot ids: slot(bo, j) = grp*48 + j*8 + (bo - grp*8).
    # Tree levels then read/write fully contiguous 24/8-slot blocks, and
    # expand runs write slot strides of 8.
    def sid(bo, j):
        grp = bo // 8
        return grp * 48 + j * 8 + (bo - grp * 8)

    runs = []   # (bo, j0, len, entry0, dentry)
    slot_key = {}
    for bo in range(1, NSEG):
        slot_key[sid(bo, 0)] = (0, bo, bo)
    for bo in range(NSEG):
        terms = segterms[bo]
        j = 0 if bo == 0 else 1   # j=0 taken by the S0 term
        assert len(terms) <= PAD - j
        bypos = dict(terms)
        for (L, e0, de) in _ap_cover(list(bypos), PAD - j):
            runs.append((bo, j, L, e0, de))
            for t in range(L):
                e = e0 + t * de
                if e in bypos:
                    slot_key[sid(bo, j + t)] = bypos.pop(e)
            j += L
        assert not bypos and j == PAD
    return runs, slot_key


RUNS, SLOT_KEY = _plan()

_CACHE = {}


def _build():
    from concourse import bacc, mybir
    import concourse.tile as tile

    f32 = mybir.dt.float32
    bf16 = mybir.dt.bfloat16
    ALU = mybir.AluOpType
    G = ROWS_PER_CORE // 128

    nc = bacc.Bacc("TRN2", target_bir_lowering=False)
    x1_d = nc.dram_tensor("x1", [ROWS_PER_CORE, D], bf16, kind="ExternalInput")
    x2_d = nc.dram_tensor("x2", [ROWS_PER_CORE, D], bf16, kind="ExternalInput")
    cg_d = nc.dram_tensor("cgslot", [128, SLOTS * 64], bf16, kind="ExternalInput")
    out_d = nc.dram_tensor("out", [ROWS_PER_CORE, D], f32, kind="ExternalOutput")

    with tile.TileContext(nc) as tc:
        with (
            tc.tile_pool(name="const", bufs=1) as constp,
            tc.tile_pool(name="scratch", bufs=1) as scrp,
            tc.tile_pool(name="big", bufs=1) as bigp,
            tc.tile_pool(name="tree", bufs=1) as treep,
        ):
            crep = constp.tile([128, SLOTS * 64], bf16)
            crep3 = crep[:].rearrange("p (s n) -> p s n", s=SLOTS)

            x1b = constp.tile([128, G, D], bf16)
            x2b = constp.tile([128, G, D], bf16)
            term = bigp.tile([128, G, NTERM, 64], bf16)
            g12 = scrp.tile([128, G, 15, 64], bf16)
            g21 = scrp.tile([128, G, 15, 64], bf16)
            g0x = scrp.tile([128, G, 15, 64], bf16)
            gx0 = scrp.tile([128, G, 15, 64], bf16)

            # inputs arrive pre-converted to bf16 (host does the cast), so
            # DMAs land directly in x1b/x2b — no staging, no Act casts.
            # Tile 0 cols 64:576 (blocks 1..8, the only per-tile product
            # inputs) ship first so DVE starts early; blocks 0 and 9..15
            # (used only by the late g-merged instructions) ship after the
            # other tiles. crep (first needed at expand) goes last.
            nc.sync.dma_start(x1b[:, 0, 64:576], x1_d[0:128, 64:576])
            nc.gpsimd.dma_start(x2b[:, 0, 64:576], x2_d[0:128, 64:576])
            for t in range(1, G):
                nc.sync.dma_start(x1b[:, t], x1_d[t*128:(t+1)*128])
                nc.gpsimd.dma_start(x2b[:, t], x2_d[t*128:(t+1)*128])
            nc.sync.dma_start(x1b[:, 0, 0:64], x1_d[0:128, 0:64])
            nc.sync.dma_start(x1b[:, 0, 576:1024], x1_d[0:128, 576:1024])
            nc.gpsimd.dma_start(x2b[:, 0, 0:64], x2_d[0:128, 0:64])
            nc.gpsimd.dma_start(x2b[:, 0, 576:1024], x2_d[0:128, 576:1024])
            nc.gpsimd.dma_start(crep[:], cg_d[:])

            for t in range(G):
                a1 = x1b[:, t].rearrange("p (b n) -> p b n", b=16)
                a2 = x2b[:, t].rearrange("p (b n) -> p b n", b=16)
                # g11 raster [3,3]
                t11 = term[:, t, E_G11:E_G11 + 9, :].rearrange(
                    "p (a b) n -> p a b n", a=3)
                nc.vector.tensor_mul(
                    t11,
                    a1[:, 1:4, :].unsqueeze(2).to_broadcast([128, 3, 3, 64]),
                    a2[:, 1:4, :].unsqueeze(1).to_broadcast([128, 3, 3, 64]))
                # g12 / g21
                v12 = g12[:, t].rearrange("p (a b) n -> p a b n", a=3)
                v21 = g21[:, t].rearrange("p (a b) n -> p a b n", a=5)
                nc.vector.tensor_mul(
                    v12,
                    a1[:, 1:4, :].unsqueeze(2).to_broadcast([128, 3, 5, 64]),
                    a2[:, 4:9, :].unsqueeze(1).to_broadcast([128, 3, 5, 64]))
                nc.vector.tensor_mul(
                    v21,
                    a1[:, 4:9, :].unsqueeze(2).to_broadcast([128, 5, 3, 64]),
                    a2[:, 1:4, :].unsqueeze(1).to_broadcast([128, 5, 3, 64]))
                # S11/A11 = g11 +- g11^T
                t11v = term[:, t, E_G11:E_G11 + 9, :].rearrange(
                    "p (a b) n -> p a b n", a=3)
                s11 = term[:, t, E_S11:E_S11 + 9, :].rearrange(
                    "p (a b) n -> p a b n", a=3)
                a11 = term[:, t, E_A11:E_A11 + 9, :].rearrange(
                    "p (a b) n -> p a b n", a=3)
                nc.vector.tensor_tensor(s11, t11v, t11v.transpose([0, 2, 1, 3]),
                                        op=ALU.add)
                nc.vector.tensor_tensor(a11, t11v, t11v.transpose([0, 2, 1, 3]),
                                        op=ALU.subtract)
                s12 = term[:, t, E_S12:E_S12 + 15, :].rearrange(
                    "p (a b) n -> p a b n", a=3)
                a12 = term[:, t, E_A12:E_A12 + 15, :].rearrange(
                    "p (a b) n -> p a b n", a=3)
                nc.vector.tensor_tensor(s12, v12, v21.transpose([0, 2, 1, 3]),
                                        op=ALU.add)
                nc.vector.tensor_tensor(a12, v12, v21.transpose([0, 2, 1, 3]),
                                        op=ALU.subtract)

            # g-merged: g00 and S0k = x1[0]*x2[blk] + x1[blk]*x2[0]
            b1v = x1b[:].rearrange("p g (b n) -> p g b n", b=16)
            b2v = x2b[:].rearrange("p g (b n) -> p g b n", b=16)
            nc.vector.tensor_mul(term[:, :, E_G00:E_G00 + 1, :],
                                 b1v[:, :, 0:1, :], b2v[:, :, 0:1, :])
            nc.vector.tensor_mul(
                g0x[:], b1v[:, :, 0:1, :].to_broadcast([128, G, 15, 64]),
                b2v[:, :, 1:16, :])
            nc.vector.tensor_mul(
                gx0[:], b1v[:, :, 1:16, :],
                b2v[:, :, 0:1, :].to_broadcast([128, G, 15, 64]))
            nc.vector.tensor_tensor(term[:, :, E_S01:E_S01 + 15, :],
                                    g0x[:], gx0[:], op=ALU.add)

            # expand+scale then tree, one 8-segment group at a time.
            # j-major slots: in-group slot = j*8 + seg_off, so tree levels
            # read fully contiguous 24/8-slot blocks.
            for grp in range(2):
                seg0 = grp * 8
                lo = grp * 48
                sp = bigp.tile([128, G, 8 * PAD, 64], bf16, tag="sp")
                t1 = treep.tile([128, G, 3 * 8, 64], bf16, tag="t1")
                u = treep.tile([128, G, 8, 64], bf16, tag="u")
                res = treep.tile([128, G, 8, 64], bf16, tag="res")
                outt = treep.tile([128, G, 8, 64], f32, tag="outt")

                # S0-family: j=0 slots, entries contiguous (E_S01 + bo - 1)
                if grp == 0:
                    nseg0, efam, sfam = 7, E_S01, 1        # segments 1..7
                else:
                    nseg0, efam, sfam = 8, E_S01 + 7, 0    # segments 8..15
                cfam = crep3[:, lo + sfam:lo + sfam + nseg0, :].unsqueeze(
                    1).to_broadcast([128, G, nseg0, 64])
                nc.vector.tensor_mul(
                    sp[:, :, sfam:sfam + nseg0, :],
                    term[:, :, efam:efam + nseg0, :], cfam)

                for (bo, j0, ln, e0, de) in RUNS:
                    if not (seg0 <= bo < seg0 + 8):
                        continue
                    if ln == 1 or de == 0:
                        gsl = term[:, :, e0:e0 + 1, :]
                        if ln > 1:
                            gsl = gsl.to_broadcast([128, G, ln, 64])
                    elif de > 0:
                        gsl = term[:, :, e0:e0 + (ln - 1) * de + 1:de, :]
                    else:
                        stop = e0 + (ln - 1) * de - 1
                        gsl = term[:, :, e0:(stop if stop >= 0 else None):de, :]
                    s0 = j0 * 8 + (bo - seg0)   # in-group slot, stride 8
                    send = s0 + (ln - 1) * 8 + 1
                    csl = crep3[:, lo + s0:lo + send:8, :].unsqueeze(
                        1).to_broadcast([128, G, ln, 64])
                    nc.vector.tensor_mul(sp[:, :, s0:send:8, :], gsl, csl)

                # tree 6->3->2->1 over j-major blocks: 3 instructions
                nc.vector.tensor_tensor(
                    t1[:], sp[:, :, 0:24, :], sp[:, :, 24:48, :], op=ALU.add)
                nc.vector.tensor_tensor(
                    u[:], t1[:, :, 0:8, :], t1[:, :, 8:16, :], op=ALU.add)
                # final add stays bf16 (2x DVE mode); the idle Activation
                # engine converts per row-group so out-DMAs start early
                nc.vector.tensor_tensor(
                    res[:], u[:], t1[:, :, 16:24, :], op=ALU.add)
                for g in range(G):
                    nc.scalar.copy(outt[:, g], res[:, g])
                    eng = nc.sync if g % 2 == 0 else nc.gpsimd
                    eng.dma_start(
                        out_d[g*128:(g+1)*128, seg0*64:(seg0 + 8)*64],
                        outt[:, g])

    nc.compile()
    return nc


def _get_nc():
    if "nc" not in _CACHE:
        _CACHE["nc"] = _build()
    return _CACHE["nc"]


def _in_maps(np_inputs):
    import ml_dtypes
    # host-side fp32 -> bf16 conversion: halves input DMA bytes and removes
    # all on-chip casts (same rounding the Activation engine would apply)
    x1 = np.asarray(np_inputs["x1"], dtype=np.float32).astype(ml_dtypes.bfloat16)
    x2 = np.asarray(np_inputs["x2"], dtype=np.float32).astype(ml_dtypes.bfloat16)
    cg = np.asarray(np_inputs["cg_tilde"], dtype=np.float32).reshape(-1, 64)
    rid1 = np.asarray(np_inputs["repids_in1"]).reshape(-1, 64)[:, 0] // 64
    rid2 = np.asarray(np_inputs["repids_in2"]).reshape(-1, 64)[:, 0] // 64
    rido = np.asarray(np_inputs["repids_out"]).reshape(-1, 64)[:, 0] // 64

    table = {}
    for k in range(cg.shape[0]):
        table[(int(rid1[k]), int(rid2[k]), int(rido[k]))] = cg[k, 0]
    cg_slot = np.zeros(SLOTS, dtype=np.float32)
    for slot, key in SLOT_KEY.items():
        cg_slot[slot] = table[key]
    cg_full = np.broadcast_to(cg_slot[:, None], (SLOTS, 64)).reshape(1, -1)
    cg_full = np.ascontiguousarray(
        np.broadcast_to(cg_full, (128, SLOTS * 64))).astype(ml_dtypes.bfloat16)

    n = x1.shape[0]
    rows = n // N_CORES
    in_maps = []
    for k in range(N_CORES):
        sl = slice(k * rows, (k + 1) * rows)
        in_maps.append({
            "x1": np.ascontiguousarray(x1[sl]),
            "x2": np.ascontiguousarray(x2[sl]),
            "cgslot": cg_full,
        })
    return in_maps


def kernel(x1, x2, cg_tilde, repids_in1, repids_in2, repids_out, out_dim):
    from concourse.bass_utils import run_bass_kernel_spmd

    nc = _get_nc()
    in_maps = _in_maps({
        "x1": x1, "x2": x2, "cg_tilde": cg_tilde, "repids_in1": repids_in1,
        "repids_in2": repids_in2, "repids_out": repids_out,
    })
    res = run_bass_kernel_spmd(nc, in_maps, core_ids=list(range(N_CORES)))
    out = np.concatenate([res.results[k]["out"] for k in range(N_CORES)], axis=0)
    return out
